# revision 9
# baseline (speedup 1.0000x reference)
"""Trainium2 Bass kernel for nn_ConvolutionalModel_44555990729204 (v2).

Math (from the reference):
    win[i,j,:]  = x windows of 4x4 (stride 4), flattened k2 = 4r+c
    rec  = relu(win @ (We@Wr) + (be@Wr + br))          # We@Wr folded: rank-16
    attn = relu(win @ Wa + ba)
    out  = x + (rec @ Ws + bs) * attn   (scattered back to windows)

v2 design (vs v1): bf16 end-to-end I/O; GPSIMD bit-shuffles folded into the
stream-transpose access patterns; the two big DVE block-transposes replaced by
ONE batched DMA xbar-transpose each way ([128,1024] -> 8x[128,128] full
transposes in a single instruction); attn broadcast x16 via a stride-0 DMA
instead of a kron matmul; single wide ACT (4 psum banks) for the rec relu;
matmuls emitted in adjacent bursts so 32-row/col PE tiles run concurrently.

Per-core layout ([2048, 1024] rows, 16 row tiles of [128, 1024] bf16):
  ST_a (DVE, per c, fused shuffle out-AP):
      xb[p=(i4..i0 r1 r0), f=4j+c] -> ta2[p=(i4 i3 | jl5), f=128jh+16*(i2i1i0)+4c+r]
  DMA-T (one instr, 8 chunks of [128,128]):
      win8[p=16*(i&7)+4c+r, f=128*(j>>5)+32*(i>>3)+(j&31)]
  matmuls: attn (K=128,M=8), m1 x4 (K=32,M=128, row-tiled), m2 x4 (K=128,M=32,
      col-tiled); rec relu: one ACT over [128, 2048] psum.
  gate: tsb = prep + bs (ACT), updwin = tsb * attn16 (GPSIMD), attn16 via DMA
      stride-0 broadcast of relu'd attn.
  inverse: DMA-T, ST_a' (DVE, fused shuffle in-AP), add x (GPSIMD), DMA out.
"""

import os
import sys

sys.path.insert(0, "/opt/trn_rl_repo")

import numpy as np

import concourse.bacc as bacc
import concourse.bass as bass
import concourse.mybir as mybir
from concourse import tile
from concourse.alu_op_type import AluOpType
from concourse.bass_utils import run_bass_kernel_spmd

F32 = mybir.dt.float32
BF16 = mybir.dt.bfloat16
RELU = mybir.ActivationFunctionType.Relu
IDENT = mybir.ActivationFunctionType.Identity

N_CORES = 8
B, H, W = 16, 1024, 1024
BPC = B // N_CORES          # images per core
ROWS = BPC * H              # 2048 rows per core
NT = ROWS // 128            # 16 row tiles per core
FH = 512                    # psum bank width in f32

# wconst column layout (matmul weights, bf16)
W2_C = slice(0, 128)        # [32, 128] replicated x4 on partitions
WA8_C = slice(128, 136)     # [128, 8]
WS2_C = slice(136, 168)     # [128, 32]
WCONST_COLS = 168
# wb column layout (biases, f32)
BCOMB2_C = slice(0, 1)      # [128, 1]
BS2_C = slice(1, 2)         # [128, 1]
BA_C = slice(2, 3)          # [8, 1] in partitions 0:8
WB_COLS = 8

# fallback knobs
KV_BCAST = os.environ.get("KV_BCAST", "ap")      # ap | loop
KV_GATE = os.environ.get("KV_GATE", "gps")       # gps | stt
KV_ATTN = os.environ.get("KV_ATTN", "dve")       # dve | act
KV_ADD = os.environ.get("KV_ADD", "gps")         # gps | dve


def _build_wconst(Wa, ba, We, be, Wr, br, Ws, bs):
    """Pack permuted weights [128, 168] bf16 + biases [128, 8] f32."""
    Wcomb = We @ Wr                       # [16, 64]
    bcomb = be @ Wr + br                  # [64]
    # partition element index e = 4r + c = k2 directly (no permutation)

    wconst = np.zeros((128, WCONST_COLS), dtype=np.float32)
    w2 = np.zeros((32, 128), dtype=np.float32)
    w2[0:16, 0:64] = Wcomb
    w2[16:32, 64:128] = Wcomb
    wconst[:, W2_C] = np.tile(w2, (4, 1))
    wa8 = np.zeros((128, 8), dtype=np.float32)
    for g in range(8):
        wa8[16 * g:16 * g + 16, g] = Wa[:, 0]
    wconst[:, WA8_C] = wa8
    ws2 = np.zeros((128, 32), dtype=np.float32)
    ws2[0:64, 0:16] = Ws
    ws2[64:128, 16:32] = Ws
    wconst[:, WS2_C] = ws2

    wb = np.zeros((128, WB_COLS), dtype=np.float32)
    wb[:, BCOMB2_C] = np.tile(bcomb, 2)[:, None]
    wb[:, BS2_C] = np.tile(bs, 8)[:, None]
    wb[0:8, BA_C] = float(ba[0])
    return wconst.astype(mybir.dt.np(BF16)), wb


def _build_nc():
    nc = bacc.Bacc()
    x = nc.dram_tensor("x", [ROWS, W], BF16, kind="ExternalInput")
    wc = nc.dram_tensor("wc", [128, WCONST_COLS], BF16, kind="ExternalInput")
    wb = nc.dram_tensor("wb", [128, WB_COLS], F32, kind="ExternalInput")
    y = nc.dram_tensor("y", [ROWS, W], BF16, kind="ExternalOutput")

    with tile.TileContext(nc) as tc:
        with (
            tc.tile_pool(name="const", bufs=1) as cpool,
            tc.tile_pool(name="io", bufs=4) as iopool,
            tc.tile_pool(name="stage", bufs=2) as spool,
            tc.tile_pool(name="win", bufs=2) as wpool,
            tc.tile_pool(name="rec", bufs=2) as recpool,
            tc.tile_pool(name="attn", bufs=2) as apool,
            tc.tile_pool(name="small", bufs=2) as smpool,
            tc.tile_pool(name="prec", bufs=1, space="PSUM") as pr_pool,
            tc.tile_pool(name="pattn", bufs=1, space="PSUM") as pa_pool,
            tc.tile_pool(name="prep", bufs=2, space="PSUM") as pp_pool,
        ):
            wconst = cpool.tile([128, WCONST_COLS], BF16)
            wbias = cpool.tile([128, WB_COLS], F32)
            scratch = cpool.tile([128, 8], F32)
            nc.sync.dma_start(wconst[0:128, :], wc[0:128, :])
            nc.vector.tensor_copy(scratch[0:1, 0:1], wconst[0:1, 0:1])
            nc.sync.dma_start(wbias[0:128, :], wb[0:128, :])
            nc.vector.tensor_copy(scratch[0:1, 2:3], wbias[0:1, 0:1])

            lhs_m1 = [wconst[32 * q:32 * q + 32, W2_C] for q in range(4)]
            lhs_attn = wconst[0:128, WA8_C]
            lhs_m2 = wconst[0:128, WS2_C]
            bias_rec = wbias[:, BCOMB2_C]
            bias_rep = wbias[:, BS2_C]
            bias_attn = wbias[0:8, BA_C]

            for t in range(int(os.environ.get("KV_NT", NT))):
                r0 = t * 128
                xb = iopool.tile([128, 1024], BF16, tag="rowin")
                nc.sync.dma_start(xb[0:64, :], x[r0:r0 + 64, :])
                nc.vector.tensor_copy(scratch[0:1, 4:5], xb[0:1, 0:1])
                nc.sync.dma_start(xb[64:128, :], x[r0 + 64:r0 + 128, :])
                nc.vector.tensor_copy(scratch[0:1, 5:6], xb[64:65, 0:1])

                # ST_a x4: in stride-4 (col c), out stride-4 (offset c) ->
                # ta2 f = 128jh + 64i2 + 16i10 + 4r + c, so each 128-chunk's
                # low 7 bits are (i2 i1 i0 r1 r0 c1 c0) for the xbar transpose
                ta2 = spool.tile([128, 1024], BF16, tag="ta2")
                rt_c = xb[:, :].rearrange("p (j c) -> p c j", c=4)
                ta2_v = ta2[:, :].rearrange("p (f c) -> p c f", c=4)
                for c in range(4):
                    nc.vector.transpose(ta2_v[:, c, :], rt_c[:, c, :])

                # batched xbar transpose: win8[a, 128m+b] = ta2[b, 128m+a]
                win8 = wpool.tile([128, 1024], BF16, tag="win8")
                win8_3 = win8[:, :].rearrange("p (m b) -> p m b", m=8, b=128)
                nc.sync.dma_start(win8_3, ta2[:, :], transpose=True)

                # attn path: 2 matmuls -> relu -> x16 partition broadcast
                pattn = pa_pool.tile([8, 1024], F32, tag="pattn")
                for h in range(2):
                    ch = slice(h * FH, (h + 1) * FH)
                    nc.tensor.matmul(
                        pattn[:, ch], lhs_attn, win8[:, ch],
                        start=True, stop=True, tile_position=(0, 0),
                    )
                attn_sb = apool.tile([8, 1024], BF16, tag="attnsb")
                if KV_ATTN == "dve":
                    nc.vector.tensor_scalar(
                        attn_sb[:, :], pattn[:, :], bias_attn, 0.0,
                        AluOpType.add, AluOpType.max,
                    )
                else:
                    nc.scalar.activation(
                        attn_sb[:, :], pattn[:, :], RELU, bias=bias_attn
                    )
                attn16 = apool.tile([128, 1024], BF16, tag="attn16")
                if KV_BCAST == "ap":
                    # contiguous write [128, 1024]; replication via stride-0
                    # read dims (partition p = 16g + k reads attn_sb[g, :])
                    src = attn_sb[:, :].rearrange(
                        "g (one f) -> g one f", one=1
                    ).broadcast_to((8, 16, 1024))
                    nc.sync.dma_start(attn16[:, :], src)
                else:
                    attn16_v = attn16[:, :].rearrange(
                        "(g k) f -> g k f", g=8, k=16
                    )
                    for k in range(16):
                        nc.sync.dma_start(attn16_v[:, k], attn_sb[:, :])

                updwin = smpool.tile([128, 1024], BF16, tag="updwin")
                for h in range(2):
                    ch = slice(h * FH, (h + 1) * FH)
                    prec = pr_pool.tile([128, 2048], F32, tag="prec")
                    for q in range(4):
                        nc.tensor.matmul(
                            prec[:, 512 * q:512 * q + 512], lhs_m1[q],
                            win8[32 * q:32 * q + 32, ch],
                            start=True, stop=True, tile_position=(32 * q, 0),
                        )
                    rec = recpool.tile([128, 2048], BF16, tag="rec")
                    nc.scalar.activation(
                        rec[:, :], prec[:, :], RELU, bias=bias_rec
                    )
                    prep = pp_pool.tile([128, FH], F32, tag="prep")
                    for q in range(4):
                        nc.tensor.matmul(
                            prep[32 * q:32 * q + 32, :], lhs_m2,
                            rec[:, 512 * q:512 * q + 512],
                            start=True, stop=True, tile_position=(0, 32 * q),
                        )
                    if KV_GATE == "stt":
                        nc.vector.scalar_tensor_tensor(
                            updwin[:, ch], prep[:, :], bias_rep,
                            attn16[:, ch], AluOpType.add, AluOpType.mult,
                        )
                    else:
                        tsb = smpool.tile([128, FH], BF16, tag="tsb")
                        nc.scalar.activation(
                            tsb[:, :], prep[:, :], IDENT, bias=bias_rep
                        )
                        nc.gpsimd.tensor_tensor(
                            updwin[:, ch], tsb[:, :], attn16[:, ch],
                            AluOpType.mult,
                        )

                # inverse: batched xbar transpose + ST_a' (fused shuffle in-AP)
                to2 = spool.tile([128, 1024], BF16, tag="to2")
                to2_3 = to2[:, :].rearrange("p (m b) -> p m b", m=8, b=128)
                nc.sync.dma_start(to2_3, updwin[:, :], transpose=True)

                updrow = spool.tile([128, 1024], BF16, tag="updrow")
                to2_v = to2[:, :].rearrange("p (f c) -> p c f", c=4)
                ur_v = updrow[:, :].rearrange("p (j c) -> p c j", c=4)
                for c in range(4):
                    nc.vector.transpose(ur_v[:, c, :], to2_v[:, c, :])

                outrow = iopool.tile([128, 1024], BF16, tag="rowout")
                if KV_ADD == "gps":
                    nc.gpsimd.tensor_tensor(
                        outrow[:, :], updrow[:, :], xb[:, :], AluOpType.add
                    )
                else:
                    nc.vector.tensor_add(outrow[:, :], updrow[:, :], xb[:, :])
                nc.sync.dma_start(y[r0:r0 + 128, :], outrow[:, :])

    if not nc.is_finalized():
        nc.finalize()
    return nc


_NC_CACHE = None


def _get_nc():
    global _NC_CACHE
    if _NC_CACHE is None:
        _NC_CACHE = _build_nc()
    return _NC_CACHE


def _f32_to_bf16_u16(a):
    """Round-to-nearest-even f32 -> bf16 (as uint16), vectorized."""
    v = np.ascontiguousarray(a, np.float32).view(np.uint32)
    r = ((v >> 16) & 1) + 0x7FFF
    return ((v + r) >> 16).astype(np.uint16)


def _bf16_to_f32(u16):
    return (u16.astype(np.uint32) << 16).view(np.float32)


def make_in_maps(x, Wa, ba, We, be, Wr, br, Ws, bs, **_ignored):
    bf16 = mybir.dt.np(BF16)
    wconst, wbias = _build_wconst(
        np.asarray(Wa, np.float32), np.asarray(ba, np.float32),
        np.asarray(We, np.float32), np.asarray(be, np.float32),
        np.asarray(Wr, np.float32), np.asarray(br, np.float32),
        np.asarray(Ws, np.float32), np.asarray(bs, np.float32),
    )
    xb16 = _f32_to_bf16_u16(np.asarray(x, np.float32)).view(bf16)
    xr = xb16.reshape(B, H, W)
    in_maps = []
    for core in range(N_CORES):
        xc = xr[core * BPC:(core + 1) * BPC].reshape(ROWS, W)
        in_maps.append({"x": xc, "wc": wconst, "wb": wbias})
    return in_maps


def kernel(x, Wa, ba, We, be, Wr, br, Ws, bs, **_ignored):
    in_maps = make_in_maps(x, Wa, ba, We, be, Wr, br, Ws, bs)
    nc = _get_nc()
    res = run_bass_kernel_spmd(nc, in_maps, list(range(N_CORES)))
    out = np.empty((B, 1, H, W), np.float32)
    for core in range(N_CORES):
        yc = np.asarray(res.results[core]["y"]).view(np.uint16)
        out[core * BPC:(core + 1) * BPC, 0] = _bf16_to_f32(yc).reshape(
            BPC, H, W
        )
    return out
